# revision 3
# baseline (speedup 1.0000x reference)
"""GRU (B=64, T=512, DIN=D=512) on 8 Trainium2 NeuronCores.

Strategy
--------
Data-parallel over batch: each core owns BL = 8 batch rows, weights are
replicated (per the sharding hint).  Per core:

1. Projection phase: xg = X @ W_g + b_g for g in {z, r, h}, computed as one
   big GEMM per gate with W stationary ([K=DIN,M=D] tiles) and X^T streaming
   ([DIN, T*BL] columns).  Output is written to DRAM scratch in *transposed*
   layout xall[d, g, t*BL+b] so the scan can consume [D-partition, batch-free]
   tiles directly.

2. Scan phase (the sequential part): state is kept transposed,
   hT [128 partitions = d%128, KT=4 k-tiles, BL=8], so that
   - the recurrent matmuls are psum[m] += U[k,m].T @ hmT[k] (U stationary,
     state streaming, output already transposed), and
   - all elementwise work (sigmoid/tanh/blend) runs on fat [128, 32] tiles.
   Per step: 48 matmuls (3 gates x 4 k-tiles x 4 m-tiles), 3 activations,
   ~7 DVE ops.  PE order r -> z -> h lets sigmoid(r) and r*h overlap the
   z-gate matmuls so U_h can start as soon as its rhs is ready.

The mask input: reference semantics are h_t = z*(m_{t-1}*h_{t-1}) + ...,
i.e. the *shifted* mask multiplies the previous state.  For the all-ones
mask (what setup_inputs produces) this is the identity, so the fast path
skips the multiply; a general path (host-broadcast shifted mask streamed
from DRAM, one extra DVE mul per step) handles arbitrary 0/1 masks.
"""

import numpy as np
from contextlib import ExitStack

import concourse.bass as bass
import concourse.bacc as bacc
import concourse.mybir as mybir
import concourse.tile as tile
from concourse.bass_utils import run_bass_kernel_spmd

FP32 = mybir.dt.float32
BF16 = mybir.dt.bfloat16
AF = mybir.ActivationFunctionType

B, T, DIN, D = 64, 512, 512, 512
NCORES = 8
BL = B // NCORES            # 8 batch rows per core
KT = DIN // 128             # 4 contraction tiles
MT = D // 128               # 4 output tiles
P = 128


def build_nc(T_=T, masked=False, u_bf16=True):
    """Build the single-core SPMD program (identical on all 8 cores)."""
    cols = T_ * BL                       # projection columns
    pcw = min(512, cols)                 # projection chunk width
    pch = cols // pcw                    # projection chunks
    tl = min(64, T_)                     # scan steps per chunk
    sch = T_ // tl                       # scan chunks
    udt = BF16 if u_bf16 else FP32

    nc = bacc.Bacc(None, target_bir_lowering=False, debug=False)

    xT = nc.dram_tensor("xT", [DIN, cols], FP32, kind="ExternalInput")
    w_lay = {g: nc.dram_tensor(f"W{g}", [P, KT * D], FP32, kind="ExternalInput")
             for g in "zrh"}
    u_lay = {g: nc.dram_tensor(f"U{g}", [P, KT * D], FP32, kind="ExternalInput")
             for g in "zrh"}
    b4 = {g: nc.dram_tensor(f"b{g}", [P, MT], FP32, kind="ExternalInput")
          for g in "zrh"}
    mb = None
    if masked:
        mb = nc.dram_tensor("mb", [T_, P, KT * BL], FP32, kind="ExternalInput")
    hT_out = nc.dram_tensor("hT_out", [D, BL], FP32, kind="ExternalOutput")

    with tile.TileContext(nc) as tc, ExitStack() as ctx:
        dram = ctx.enter_context(tc.tile_pool(name="dram", bufs=1, space="DRAM"))
        xall = dram.tile([D, 3, cols], FP32)

        upool = ctx.enter_context(tc.tile_pool(name="upool", bufs=1))
        u_sb = {}
        for g in "zrh":
            if u_bf16:
                stage = upool.tile([P, KT * D], FP32, tag="ustage", name="ustage")
                nc.sync.dma_start(stage[:], u_lay[g][:])
                u_sb[g] = upool.tile([P, KT * D], BF16, tag=f"u{g}", name=f"u{g}")
                nc.vector.tensor_copy(u_sb[g][:], stage[:])
            else:
                u_sb[g] = upool.tile([P, KT * D], FP32, tag=f"u{g}", name=f"u{g}")
                nc.sync.dma_start(u_sb[g][:], u_lay[g][:])

        # ---------------- projection phase ----------------
        with (
            tc.tile_pool(name="wpool", bufs=1) as wpool,
            tc.tile_pool(name="xtp", bufs=2 * KT) as xtp,
            tc.tile_pool(name="pproj", bufs=2, space="PSUM") as pproj,
            tc.tile_pool(name="evp", bufs=4) as evp,
            tc.tile_pool(name="bp", bufs=1) as bp,
        ):
            w_sb = {}
            b_sb = {}
            for g in "zrh":
                w_sb[g] = wpool.tile([P, KT * D], FP32, tag=f"w{g}", name=f"w{g}")
                nc.sync.dma_start(w_sb[g][:], w_lay[g][:])
                b_sb[g] = bp.tile([P, MT], FP32, tag=f"b{g}", name=f"b{g}")
                nc.sync.dma_start(b_sb[g][:], b4[g][:])

            for c in range(pch):
                xt_tiles = []
                for kk in range(KT):
                    xt = xtp.tile([P, pcw], FP32, tag="xt")
                    nc.sync.dma_start(
                        xt[:], xT[kk * P:(kk + 1) * P, c * pcw:(c + 1) * pcw])
                    xt_tiles.append(xt)
                for gi, g in enumerate("zrh"):
                    for m in range(MT):
                        ps = pproj.tile([P, pcw], FP32, tag="pp")
                        for kk in range(KT):
                            nc.tensor.matmul(
                                ps[:],
                                w_sb[g][:, kk * D + m * P: kk * D + (m + 1) * P],
                                xt_tiles[kk][:],
                                start=(kk == 0), stop=(kk == KT - 1))
                        ev = evp.tile([P, pcw], FP32, tag="ev")
                        nc.vector.tensor_scalar_add(ev[:], ps[:], b_sb[g][:, m:m + 1])
                        nc.sync.dma_start(
                            xall[m * P:(m + 1) * P, gi, c * pcw:(c + 1) * pcw], ev[:])

        # ---------------- scan phase ----------------
        with (
            tc.tile_pool(name="xsb", bufs=2) as xpool,
            tc.tile_pool(name="psc", bufs=2, space="PSUM") as psc,
            tc.tile_pool(name="sm", bufs=3) as sm,
            tc.tile_pool(name="mbp", bufs=2) as mbp,
        ):
            h_prev = sm.tile([P, KT, BL], FP32, tag="h")
            nc.vector.memset(h_prev[:], 0.0)
            hm_bf_init = None
            if u_bf16:
                hm_bf_init = sm.tile([P, KT, BL], BF16, tag="hmbf")
                nc.vector.memset(hm_bf_init[:], 0.0)
            hm_bf = hm_bf_init

            for c in range(sch):
                xsb = xpool.tile([P, 3, KT, tl, BL], FP32, tag="x")
                for gi in range(3):
                    for kk in range(KT):
                        nc.sync.dma_start(
                            xsb[:, gi, kk],
                            xall[kk * P:(kk + 1) * P, gi,
                                 c * tl * BL:(c + 1) * tl * BL])
                if masked:
                    mb_sb = mbp.tile([P, tl, KT * BL], FP32, tag="m")
                    nc.sync.dma_start(
                        mb_sb[:],
                        mb[c * tl:(c + 1) * tl].rearrange("t p x -> p t x"))

                for t in range(tl):
                    if masked:
                        hm = sm.tile([P, KT, BL], FP32, tag="hm")
                        nc.vector.tensor_mul(
                            hm[:], h_prev[:],
                            mb_sb[:, t].rearrange("p (k b) -> p k b", k=KT))
                    else:
                        hm = h_prev
                    if u_bf16:
                        if masked or hm_bf is None:
                            hm_bf = sm.tile([P, KT, BL], BF16, tag="hmbf")
                            nc.vector.tensor_copy(hm_bf[:], hm[:])
                        rhs_hm = hm_bf
                    else:
                        rhs_hm = hm

                    # r gate matmuls
                    ps_r = psc.tile([P, KT, BL], FP32, tag="pr")
                    for m in range(MT):
                        for kk in range(KT):
                            nc.tensor.matmul(
                                ps_r[:, m],
                                u_sb["r"][:, kk * D + m * P: kk * D + (m + 1) * P],
                                rhs_hm[:, kk],
                                start=(kk == 0), stop=(kk == KT - 1))
                    rpre = sm.tile([P, KT, BL], FP32, tag="rpre")
                    nc.vector.tensor_add(rpre[:], ps_r[:], xsb[:, 1, :, t])
                    r_sb = sm.tile([P, KT, BL], FP32, tag="r")
                    nc.scalar.activation(r_sb[:], rpre[:], AF.Sigmoid)
                    rhm = sm.tile([P, KT, BL], udt, tag="rhm")
                    nc.vector.tensor_mul(rhm[:], r_sb[:], hm[:])

                    # z gate matmuls (overlap sigmoid(r) above)
                    ps_z = psc.tile([P, KT, BL], FP32, tag="pz")
                    for m in range(MT):
                        for kk in range(KT):
                            nc.tensor.matmul(
                                ps_z[:, m],
                                u_sb["z"][:, kk * D + m * P: kk * D + (m + 1) * P],
                                rhs_hm[:, kk],
                                start=(kk == 0), stop=(kk == KT - 1))

                    # h candidate matmuls
                    ps_h = psc.tile([P, KT, BL], FP32, tag="ph")
                    for m in range(MT):
                        for kk in range(KT):
                            nc.tensor.matmul(
                                ps_h[:, m],
                                u_sb["h"][:, kk * D + m * P: kk * D + (m + 1) * P],
                                rhm[:, kk],
                                start=(kk == 0), stop=(kk == KT - 1))

                    zpre = sm.tile([P, KT, BL], FP32, tag="zpre")
                    nc.vector.tensor_add(zpre[:], ps_z[:], xsb[:, 0, :, t])
                    z_sb = sm.tile([P, KT, BL], FP32, tag="z")
                    nc.scalar.activation(z_sb[:], zpre[:], AF.Sigmoid)

                    hpre = sm.tile([P, KT, BL], FP32, tag="hpre")
                    nc.vector.tensor_add(hpre[:], ps_h[:], xsb[:, 2, :, t])
                    hh = sm.tile([P, KT, BL], FP32, tag="hh")
                    nc.scalar.activation(hh[:], hpre[:], AF.Tanh)

                    # h_new = hh + z * (hm - hh)
                    dd = sm.tile([P, KT, BL], FP32, tag="dd")
                    nc.vector.tensor_sub(dd[:], hm[:], hh[:])
                    t1 = sm.tile([P, KT, BL], FP32, tag="t1")
                    nc.vector.tensor_mul(t1[:], z_sb[:], dd[:])
                    h_new = sm.tile([P, KT, BL], FP32, tag="h")
                    nc.vector.tensor_add(h_new[:], hh[:], t1[:])
                    h_prev = h_new
                    if u_bf16 and not masked:
                        hm_bf = sm.tile([P, KT, BL], BF16, tag="hmbf")
                        nc.vector.tensor_copy(hm_bf[:], h_new[:])

            for kk in range(KT):
                nc.sync.dma_start(hT_out[kk * P:(kk + 1) * P, :], h_prev[:, kk])

    nc.compile()
    return nc


_NC_CACHE = {}


def _get_nc(masked, u_bf16=True):
    key = (masked, u_bf16)
    if key not in _NC_CACHE:
        _NC_CACHE[key] = build_nc(T, masked=masked, u_bf16=u_bf16)
    return _NC_CACHE[key]


def _w_layout(w):
    # [DIN, D] -> [128, KT*D] with lay[p, kk*D + j] = w[kk*128 + p, j]
    return np.ascontiguousarray(
        w.reshape(KT, P, D).transpose(1, 0, 2).reshape(P, KT * D), dtype=np.float32)


def _b_layout(b):
    return np.ascontiguousarray(b.reshape(MT, P).T, dtype=np.float32)


def kernel(X, W_z, U_z, b_z, W_r, U_r, b_r, W_h, U_h, b_h, mask):
    X = np.asarray(X, dtype=np.float32)
    mask = np.asarray(mask)
    masked = not bool(np.all(mask[:, :T - 1] == 1))

    nc = _get_nc(masked)

    shared = {}
    for g, w, u, b in (("z", W_z, U_z, b_z), ("r", W_r, U_r, b_r),
                       ("h", W_h, U_h, b_h)):
        shared[f"W{g}"] = _w_layout(np.asarray(w, dtype=np.float32))
        shared[f"U{g}"] = _w_layout(np.asarray(u, dtype=np.float32))
        shared[f"b{g}"] = _b_layout(np.asarray(b, dtype=np.float32))

    in_maps = []
    for c in range(NCORES):
        bsl = slice(c * BL, (c + 1) * BL)
        m = dict(shared)
        m["xT"] = np.ascontiguousarray(
            X[bsl].transpose(2, 1, 0).reshape(DIN, T * BL))
        if masked:
            msh = np.zeros((T, BL), dtype=np.float32)
            msh[1:] = mask[bsl, :T - 1].T.astype(np.float32)
            m["mb"] = np.ascontiguousarray(
                np.tile(msh[:, None, :], (1, P, KT)))
        in_maps.append(m)

    res = run_bass_kernel_spmd(nc, in_maps, core_ids=list(range(NCORES)))
    out = np.empty((B, D), dtype=np.float32)
    for c in range(NCORES):
        out[c * BL:(c + 1) * BL] = res.results[c]["hT_out"].T
    return out


# revision 6
# speedup vs baseline: 1.2759x; 1.2759x over previous
"""GRU (B=64, T=512, DIN=D=512) on 8 Trainium2 NeuronCores.

Strategy
--------
Data-parallel over batch: each core owns BL = 8 batch rows, weights are
replicated (per the sharding hint).  Per core:

1. Projection phase: xg = X @ W_g + b_g for g in {z, r, h}, computed as one
   big GEMM per gate with W stationary ([K=DIN,M=D] tiles) and X^T streaming
   ([DIN, T*BL] columns).  Output goes to per-chunk DRAM scratch tiles in
   *transposed* layout xc[d, g, t*BL+b] so the scan can consume
   [D-partition, batch-free] tiles directly and chunk c of the scan only
   depends on projection chunk c (phases pipeline).

2. Scan phase (the sequential part): state is kept transposed,
   hT [128 partitions = d%128, KT=4 k-tiles, BL=8], so that
   - the recurrent matmuls are psum[m] += U[k,m].T @ hmT[k] (U stationary,
     state streaming, output already transposed), and
   - all elementwise work (sigmoid/tanh/blend) runs on fat [128, 32] tiles.
   The x-projection term is accumulated into PSUM by an identity matmul
   (start=True) so the activations read PSUM directly - no DVE pre-adds.
   The update gate is computed as zc = sigmoid(-zpre) = 1 - z (free affine
   scale=-1 on the ACT op), which turns the blend into
       h = (hm - zc*hm) + zc*hh
   where (hm - zc*hm) is computed off the critical path; only zc*hh and the
   final add sit between tanh and the next step's matmuls.

The mask input: reference semantics are h_t = z*(m_{t-1}*h_{t-1}) + ...,
i.e. the *shifted* mask multiplies the previous state.  For the all-ones
mask (what setup_inputs produces) this is the identity, so the fast path
skips the multiply; a general path (host-broadcast shifted mask streamed
from DRAM, one extra DVE mul per step) handles arbitrary 0/1 masks.
"""

import numpy as np
from contextlib import ExitStack

import concourse.bass as bass
import concourse.bacc as bacc
import concourse.mybir as mybir
import concourse.tile as tile
from concourse.bass_utils import run_bass_kernel_spmd

FP32 = mybir.dt.float32
BF16 = mybir.dt.bfloat16
AF = mybir.ActivationFunctionType

B, T, DIN, D = 64, 512, 512, 512
NCORES = 8
BL = B // NCORES            # 8 batch rows per core
KT = DIN // 128             # 4 contraction tiles
MT = D // 128               # 4 output tiles
P = 128


def build_nc(T_=T, masked=False, use_bf16=True):
    """Build the single-core SPMD program (identical on all 8 cores)."""
    tl = min(64, T_)                     # scan steps per chunk
    sch = T_ // tl                       # chunks (projection chunk == scan chunk)
    pcw = tl * BL                        # chunk width in columns (512)
    ldt = BF16 if use_bf16 else FP32     # low-precision dtype

    nc = bacc.Bacc(None, target_bir_lowering=False, debug=False)

    xT = nc.dram_tensor("xT", [DIN, T_ * BL], FP32, kind="ExternalInput")
    w_lay = {g: nc.dram_tensor(f"W{g}", [P, KT * D], FP32, kind="ExternalInput")
             for g in "zrh"}
    u_lay = {g: nc.dram_tensor(f"U{g}", [P, KT * D], FP32, kind="ExternalInput")
             for g in "zrh"}
    b4 = {g: nc.dram_tensor(f"b{g}", [P, MT], FP32, kind="ExternalInput")
          for g in "zrh"}
    eye_d = nc.dram_tensor("eye", [P, P], FP32, kind="ExternalInput")
    mb = None
    if masked:
        mb = nc.dram_tensor("mb", [T_, P, KT * BL], FP32, kind="ExternalInput")
    hT_out = nc.dram_tensor("hT_out", [D, BL], FP32, kind="ExternalOutput")

    with tile.TileContext(nc) as tc, ExitStack() as ctx:
        dram = ctx.enter_context(tc.tile_pool(name="dram", bufs=1, space="DRAM"))
        # one DRAM scratch tile per chunk so scan chunk c only waits on
        # projection chunk c
        xc_tiles = [dram.tile([D, 3, pcw], ldt, tag=f"xc{c}", name=f"xc{c}")
                    for c in range(sch)]

        upool = ctx.enter_context(tc.tile_pool(name="upool", bufs=1))
        u_sb = {}
        eye_sb = upool.tile([P, P], ldt, tag="eye", name="eye")
        if use_bf16:
            eye_stage = upool.tile([P, P], FP32, tag="eyestage", name="eyestage")
            nc.sync.dma_start(eye_stage[:], eye_d[:])
            nc.vector.tensor_copy(eye_sb[:], eye_stage[:])
        else:
            nc.sync.dma_start(eye_sb[:], eye_d[:])
        for g in "zrh":
            if use_bf16:
                stage = upool.tile([P, KT * D], FP32, tag="ustage", name="ustage")
                nc.sync.dma_start(stage[:], u_lay[g][:])
                u_sb[g] = upool.tile([P, KT * D], BF16, tag=f"u{g}", name=f"u{g}")
                nc.vector.tensor_copy(u_sb[g][:], stage[:])
            else:
                u_sb[g] = upool.tile([P, KT * D], FP32, tag=f"u{g}", name=f"u{g}")
                nc.sync.dma_start(u_sb[g][:], u_lay[g][:])

        # ---------------- projection phase ----------------
        with (
            tc.tile_pool(name="wpool", bufs=1) as wpool,
            tc.tile_pool(name="xtp", bufs=2 * KT) as xtp,
            tc.tile_pool(name="pproj", bufs=2, space="PSUM") as pproj,
            tc.tile_pool(name="evp", bufs=4) as evp,
            tc.tile_pool(name="bp", bufs=1) as bp,
        ):
            w_sb = {}
            b_sb = {}
            for g in "zrh":
                w_sb[g] = wpool.tile([P, KT * D], FP32, tag=f"w{g}", name=f"w{g}")
                nc.sync.dma_start(w_sb[g][:], w_lay[g][:])
                b_sb[g] = bp.tile([P, MT], FP32, tag=f"b{g}", name=f"b{g}")
                nc.sync.dma_start(b_sb[g][:], b4[g][:])

            for c in range(sch):
                xt_tiles = []
                for kk in range(KT):
                    xt = xtp.tile([P, pcw], FP32, tag="xt")
                    nc.sync.dma_start(
                        xt[:], xT[kk * P:(kk + 1) * P, c * pcw:(c + 1) * pcw])
                    xt_tiles.append(xt)
                for gi, g in enumerate("zrh"):
                    for m in range(MT):
                        ps = pproj.tile([P, pcw], FP32, tag="pp")
                        for kk in range(KT):
                            nc.tensor.matmul(
                                ps[:],
                                w_sb[g][:, kk * D + m * P: kk * D + (m + 1) * P],
                                xt_tiles[kk][:],
                                start=(kk == 0), stop=(kk == KT - 1))
                        ev = evp.tile([P, pcw], ldt, tag="ev")
                        nc.vector.tensor_scalar_add(ev[:], ps[:], b_sb[g][:, m:m + 1])
                        nc.sync.dma_start(
                            xc_tiles[c][m * P:(m + 1) * P, gi, :], ev[:])

        # ---------------- scan phase ----------------
        with (
            tc.tile_pool(name="xsb", bufs=2) as xpool,
            tc.tile_pool(name="psc", bufs=2, space="PSUM") as psc,
            tc.tile_pool(name="sm", bufs=3) as sm,
            tc.tile_pool(name="mbp", bufs=2) as mbp,
        ):
            h_prev = sm.tile([P, KT, BL], ldt, tag="h", name="h0")
            nc.vector.memset(h_prev[:], 0.0)

            def gate_mms(psum, g, rhs, xv):
                # identity matmul accumulates the x-projection into PSUM
                # first (start=True, one MM covers all 4 m-regions); it has
                # no data deps beyond the chunk DMA, so PE can issue it
                # while waiting for rhs.
                nc.tensor.matmul(psum[:], eye_sb[:], xv[:],
                                 start=True, stop=False)
                for m in range(MT):
                    for kk in range(KT):
                        nc.tensor.matmul(
                            psum[:, m],
                            u_sb[g][:, kk * D + m * P: kk * D + (m + 1) * P],
                            rhs[:, kk],
                            start=False,
                            stop=(m == MT - 1 and kk == KT - 1))

            for c in range(sch):
                xsb = xpool.tile([P, 3, KT, tl, BL], ldt, tag="x")
                for gi in range(3):
                    for kk in range(KT):
                        nc.sync.dma_start(
                            xsb[:, gi, kk],
                            xc_tiles[c][kk * P:(kk + 1) * P, gi, :])
                if masked:
                    mb_sb = mbp.tile([P, tl, KT * BL], FP32, tag="m")
                    nc.sync.dma_start(
                        mb_sb[:],
                        mb[c * tl:(c + 1) * tl].rearrange("t p x -> p t x"))

                for t in range(tl):
                    if masked:
                        hm = sm.tile([P, KT, BL], ldt, tag="hm")
                        nc.vector.tensor_mul(
                            hm[:], h_prev[:],
                            mb_sb[:, t].rearrange("p (k b) -> p k b", k=KT))
                    else:
                        hm = h_prev

                    # r gate
                    ps_r = psc.tile([P, KT, BL], FP32, tag="pr")
                    gate_mms(ps_r, "r", hm, xsb[:, 1, :, t])
                    r_sb = sm.tile([P, KT, BL], ldt, tag="r")
                    nc.scalar.activation(r_sb[:], ps_r[:], AF.Sigmoid)
                    rhm = sm.tile([P, KT, BL], ldt, tag="rhm")
                    nc.vector.tensor_mul(rhm[:], r_sb[:], hm[:])

                    # z gate (complement): zc = 1 - z = sigmoid(-zpre)
                    ps_z = psc.tile([P, KT, BL], FP32, tag="pz")
                    gate_mms(ps_z, "z", hm, xsb[:, 0, :, t])
                    zc = sm.tile([P, KT, BL], ldt, tag="zc")
                    nc.scalar.activation(zc[:], ps_z[:], AF.Sigmoid, scale=-1.0)
                    # off-critical-path part of the blend: c1 = hm - zc*hm
                    zchm = sm.tile([P, KT, BL], ldt, tag="zchm")
                    nc.vector.tensor_mul(zchm[:], zc[:], hm[:])
                    c1 = sm.tile([P, KT, BL], ldt, tag="c1")
                    nc.vector.tensor_sub(c1[:], hm[:], zchm[:])

                    # h candidate
                    ps_h = psc.tile([P, KT, BL], FP32, tag="ph")
                    gate_mms(ps_h, "h", rhm, xsb[:, 2, :, t])
                    hh = sm.tile([P, KT, BL], ldt, tag="hh")
                    nc.scalar.activation(hh[:], ps_h[:], AF.Tanh)

                    # critical tail: h = c1 + zc*hh
                    b2 = sm.tile([P, KT, BL], ldt, tag="b2")
                    nc.vector.tensor_mul(b2[:], zc[:], hh[:])
                    h_new = sm.tile([P, KT, BL], ldt, tag="h")
                    nc.vector.tensor_add(h_new[:], c1[:], b2[:])
                    h_prev = h_new

            hout = sm.tile([P, KT, BL], FP32, tag="hout", name="hout")
            nc.vector.tensor_copy(hout[:], h_prev[:])
            for kk in range(KT):
                nc.sync.dma_start(hT_out[kk * P:(kk + 1) * P, :], hout[:, kk])

    nc.compile()
    return nc


_NC_CACHE = {}


def _get_nc(masked, use_bf16=True):
    key = (masked, use_bf16)
    if key not in _NC_CACHE:
        _NC_CACHE[key] = build_nc(T, masked=masked, use_bf16=use_bf16)
    return _NC_CACHE[key]


def _w_layout(w):
    # [DIN, D] -> [128, KT*D] with lay[p, kk*D + j] = w[kk*128 + p, j]
    return np.ascontiguousarray(
        w.reshape(KT, P, D).transpose(1, 0, 2).reshape(P, KT * D), dtype=np.float32)


def _b_layout(b):
    return np.ascontiguousarray(b.reshape(MT, P).T, dtype=np.float32)


def make_in_maps(X, W_z, U_z, b_z, W_r, U_r, b_r, W_h, U_h, b_h, mask,
                 masked):
    X = np.asarray(X, dtype=np.float32)
    shared = {"eye": np.eye(P, dtype=np.float32)}
    for g, w, u, b in (("z", W_z, U_z, b_z), ("r", W_r, U_r, b_r),
                       ("h", W_h, U_h, b_h)):
        shared[f"W{g}"] = _w_layout(np.asarray(w, dtype=np.float32))
        shared[f"U{g}"] = _w_layout(np.asarray(u, dtype=np.float32))
        shared[f"b{g}"] = _b_layout(np.asarray(b, dtype=np.float32))

    in_maps = []
    for c in range(NCORES):
        bsl = slice(c * BL, (c + 1) * BL)
        m = dict(shared)
        m["xT"] = np.ascontiguousarray(
            X[bsl].transpose(2, 1, 0).reshape(DIN, T * BL))
        if masked:
            msh = np.zeros((T, BL), dtype=np.float32)
            msh[1:] = np.asarray(mask)[bsl, :T - 1].T.astype(np.float32)
            m["mb"] = np.ascontiguousarray(
                np.tile(msh[:, None, :], (1, P, KT)))
        in_maps.append(m)
    return in_maps


def kernel(X, W_z, U_z, b_z, W_r, U_r, b_r, W_h, U_h, b_h, mask):
    mask = np.asarray(mask)
    masked = not bool(np.all(mask[:, :T - 1] == 1))
    nc = _get_nc(masked)
    in_maps = make_in_maps(X, W_z, U_z, b_z, W_r, U_r, b_r, W_h, U_h, b_h,
                           mask, masked)
    res = run_bass_kernel_spmd(nc, in_maps, core_ids=list(range(NCORES)))
    out = np.empty((B, D), dtype=np.float32)
    for c in range(NCORES):
        out[c * BL:(c + 1) * BL] = res.results[c]["hT_out"].T
    return out


# revision 7
# speedup vs baseline: 1.3097x; 1.0265x over previous
"""GRU (B=64, T=512, DIN=D=512) on 8 Trainium2 NeuronCores.

Strategy
--------
Data-parallel over batch: each core owns BL = 8 batch rows, weights are
replicated (per the sharding hint).  Per core:

1. Projection phase: xg = X @ W_g + b_g for g in {z, r, h} as GEMMs with W
   stationary and X^T streaming, written straight into an SBUF-resident
   pre-activation buffer xall[p, g, m, t*BL+b] (bf16, ~96KB/partition) by
   ScalarE Identity-with-bias ops.  Projection chunks 0-1 run as a
   prologue; the remaining chunks are interleaved into the scan's PE idle
   windows (chunk c+2 is emitted during scan chunk c), so projection time
   is almost entirely hidden.

2. Scan phase (the sequential part): state is kept transposed,
   hT [128 partitions = d%128, KT=4 k-tiles, BL=8], so that
   - the recurrent matmuls are psum[m] += U[k,m].T @ hmT[k] (U stationary,
     state streaming, output already transposed), and
   - all elementwise work (sigmoid/tanh/blend) runs on fat [128, 32] tiles.
   The x-projection term is accumulated into PSUM by an identity matmul
   (start=True) so the activations read PSUM directly - no DVE pre-adds.
   The update gate is computed as zc = sigmoid(-zpre) = 1 - z (free affine
   scale=-1 on the ACT op), which turns the blend into
       h = (hm - zc*hm) + zc*hh
   where (hm - zc*hm) is computed off the critical path; only zc*hh and
   the final add sit between tanh and the next step's matmuls, and those
   run in k-halves so the next step's k0/k1 matmuls start after half the
   blend.

The mask input: reference semantics are h_t = z*(m_{t-1}*h_{t-1}) + ...,
i.e. the *shifted* mask multiplies the previous state.  For the all-ones
mask (what setup_inputs produces) this is the identity, so the fast path
skips the multiply; a general path (host-broadcast shifted mask streamed
from DRAM, one extra DVE mul per step) handles arbitrary 0/1 masks.
"""

import numpy as np
from contextlib import ExitStack

import concourse.bass as bass
import concourse.bacc as bacc
import concourse.mybir as mybir
import concourse.tile as tile
from concourse.bass_utils import run_bass_kernel_spmd

FP32 = mybir.dt.float32
BF16 = mybir.dt.bfloat16
AF = mybir.ActivationFunctionType

B, T, DIN, D = 64, 512, 512, 512
NCORES = 8
BL = B // NCORES            # 8 batch rows per core
KT = DIN // 128             # 4 contraction tiles
MT = D // 128               # 4 output tiles
P = 128


def build_nc(T_=T, masked=False, use_bf16=True):
    """Build the single-core SPMD program (identical on all 8 cores)."""
    tl = min(64, T_)                     # steps per chunk
    sch = T_ // tl                       # chunks
    pcw = tl * BL                        # chunk width in columns (512)
    ldt = BF16 if use_bf16 else FP32     # low-precision dtype

    nc = bacc.Bacc(None, target_bir_lowering=False, debug=False)

    xT = nc.dram_tensor("xT", [DIN, T_ * BL], FP32, kind="ExternalInput")
    w_lay = {g: nc.dram_tensor(f"W{g}", [P, KT * D], FP32, kind="ExternalInput")
             for g in "zrh"}
    u_lay = {g: nc.dram_tensor(f"U{g}", [P, KT * D], FP32, kind="ExternalInput")
             for g in "zrh"}
    b4 = {g: nc.dram_tensor(f"b{g}", [P, MT], FP32, kind="ExternalInput")
          for g in "zrh"}
    eye_d = nc.dram_tensor("eye", [P, P], FP32, kind="ExternalInput")
    mb = None
    if masked:
        mb = nc.dram_tensor("mb", [T_, P, KT * BL], FP32, kind="ExternalInput")
    hT_out = nc.dram_tensor("hT_out", [D, BL], FP32, kind="ExternalOutput")

    with tile.TileContext(nc) as tc, ExitStack() as ctx:
        upool = ctx.enter_context(tc.tile_pool(name="upool", bufs=1))
        wpool = ctx.enter_context(tc.tile_pool(name="wpool", bufs=1))
        bp = ctx.enter_context(tc.tile_pool(name="bp", bufs=1))
        xap = ctx.enter_context(tc.tile_pool(name="xap", bufs=1))
        xtp = ctx.enter_context(tc.tile_pool(name="xtp", bufs=2 * KT))
        pproj = ctx.enter_context(
            tc.tile_pool(name="pproj", bufs=2, space="PSUM"))
        psc = ctx.enter_context(tc.tile_pool(name="psc", bufs=2, space="PSUM"))
        sm = ctx.enter_context(tc.tile_pool(name="sm", bufs=3))
        mbp = ctx.enter_context(tc.tile_pool(name="mbp", bufs=2))

        u_sb = {}
        eye_sb = upool.tile([P, P], ldt, tag="eye", name="eye")
        if use_bf16:
            eye_stage = upool.tile([P, P], FP32, tag="eyestage", name="eyestage")
            nc.sync.dma_start(eye_stage[:], eye_d[:])
            nc.vector.tensor_copy(eye_sb[:], eye_stage[:])
        else:
            nc.sync.dma_start(eye_sb[:], eye_d[:])
        w_sb = {}
        b_sb = {}
        for g in "zrh":
            if use_bf16:
                stage = upool.tile([P, KT * D], FP32, tag="ustage", name="ustage")
                nc.sync.dma_start(stage[:], u_lay[g][:])
                u_sb[g] = upool.tile([P, KT * D], BF16, tag=f"u{g}", name=f"u{g}")
                nc.vector.tensor_copy(u_sb[g][:], stage[:])
            else:
                u_sb[g] = upool.tile([P, KT * D], FP32, tag=f"u{g}", name=f"u{g}")
                nc.sync.dma_start(u_sb[g][:], u_lay[g][:])
            w_sb[g] = wpool.tile([P, KT * D], FP32, tag=f"w{g}", name=f"w{g}")
            nc.sync.dma_start(w_sb[g][:], w_lay[g][:])
            b_sb[g] = bp.tile([P, MT], FP32, tag=f"b{g}", name=f"b{g}")
            nc.sync.dma_start(b_sb[g][:], b4[g][:])

        # SBUF-resident pre-activations: [p, gate, m-tile, t*BL+b]
        xall = xap.tile([P, 3, KT, T_ * BL], ldt, tag="xall", name="xall")

        gate_i = {"z": 0, "r": 1, "h": 2}
        xt_tiles = {}

        def emit_xt_dmas(c):
            tiles = []
            for kk in range(KT):
                xt = xtp.tile([P, pcw], FP32, tag="xt", name=f"xt{c}_{kk}")
                nc.sync.dma_start(
                    xt[:], xT[kk * P:(kk + 1) * P, c * pcw:(c + 1) * pcw])
                tiles.append(xt)
            xt_tiles[c] = tiles

        def emit_proj_unit(c, g, m):
            ps = pproj.tile([P, pcw], FP32, tag="pp", name=f"pp{c}{g}{m}")
            for kk in range(KT):
                nc.tensor.matmul(
                    ps[:],
                    w_sb[g][:, kk * D + m * P: kk * D + (m + 1) * P],
                    xt_tiles[c][kk][:],
                    start=(kk == 0), stop=(kk == KT - 1))
            nc.scalar.activation(
                xall[:, gate_i[g], m, c * pcw:(c + 1) * pcw], ps[:],
                AF.Identity, bias=b_sb[g][:, m:m + 1])

        proj_units = [(c, g, m) for c in range(sch)
                      for g in "zrh" for m in range(MT)]
        # prologue: chunks 0 and 1 (or everything if <=2 chunks)
        n_pro = min(sch, 2)
        for c in range(n_pro):
            emit_xt_dmas(c)
        for c, g, m in [u for u in proj_units if u[0] < n_pro]:
            emit_proj_unit(c, g, m)
        rest = [u for u in proj_units if u[0] >= n_pro]

        def gate_mms(psum, g, rhs, xv):
            # identity matmul accumulates the x-projection into PSUM first
            # (start=True, one MM covers all 4 m-regions); it has no data
            # deps beyond the projection, so PE can issue it while waiting
            # for rhs.
            nc.tensor.matmul(psum[:], eye_sb[:], xv[:],
                             start=True, stop=False)
            for m in range(MT):
                for kk in range(KT):
                    nc.tensor.matmul(
                        psum[:, m],
                        u_sb[g][:, kk * D + m * P: kk * D + (m + 1) * P],
                        rhs[:, kk],
                        start=False,
                        stop=(m == MT - 1 and kk == KT - 1))

        h_prev = sm.tile([P, KT, BL], ldt, tag="h", name="h0")
        nc.vector.memset(h_prev[:], 0.0)

        for t in range(T_):
            c = t // tl
            ti = t % tl
            if ti == 0:
                if c + 2 < sch:
                    emit_xt_dmas(c + 2)
                if masked:
                    mb_sb = mbp.tile([P, tl, KT * BL], FP32, tag="m",
                                     name=f"mb{c}")
                    nc.sync.dma_start(
                        mb_sb[:],
                        mb[c * tl:(c + 1) * tl].rearrange("t p x -> p t x"))

            if masked:
                hm = sm.tile([P, KT, BL], ldt, tag="hm")
                nc.vector.tensor_mul(
                    hm[:], h_prev[:],
                    mb_sb[:, ti].rearrange("p (k b) -> p k b", k=KT))
            else:
                hm = h_prev

            xv = xall[:, :, :, t * BL:(t + 1) * BL]

            # r gate
            ps_r = psc.tile([P, KT, BL], FP32, tag="pr")
            gate_mms(ps_r, "r", hm, xv[:, 1])
            r_sb = sm.tile([P, KT, BL], ldt, tag="r")
            nc.scalar.activation(r_sb[:], ps_r[:], AF.Sigmoid)
            rhm = sm.tile([P, KT, BL], ldt, tag="rhm")
            nc.vector.tensor_mul(rhm[:], r_sb[:], hm[:])

            # z gate (complement): zc = 1 - z = sigmoid(-zpre)
            ps_z = psc.tile([P, KT, BL], FP32, tag="pz")
            gate_mms(ps_z, "z", hm, xv[:, 0])
            zc = sm.tile([P, KT, BL], ldt, tag="zc")
            nc.scalar.activation(zc[:], ps_z[:], AF.Sigmoid, scale=-1.0)
            # off-critical-path part of the blend: c1 = hm - zc*hm
            zchm = sm.tile([P, KT, BL], ldt, tag="zchm")
            nc.vector.tensor_mul(zchm[:], zc[:], hm[:])
            c1 = sm.tile([P, KT, BL], ldt, tag="c1")
            nc.vector.tensor_sub(c1[:], hm[:], zchm[:])

            # h candidate
            ps_h = psc.tile([P, KT, BL], FP32, tag="ph")
            gate_mms(ps_h, "h", rhm, xv[:, 2])

            # interleave one hidden projection unit into this step's tail
            # window (PE would otherwise idle here)
            if rest and ti % 5 == 0 and (ti // 5) < 12:
                emit_proj_unit(*rest.pop(0))

            # critical tail in k-halves: h = c1 + zc*hh; the next step's
            # k0/k1 matmuls only need the first half of h.
            hh = sm.tile([P, KT, BL], ldt, tag="hh")
            b2 = sm.tile([P, KT, BL], ldt, tag="b2")
            h_new = sm.tile([P, KT, BL], ldt, tag="h")
            for hf in range(2):
                sl = slice(2 * hf, 2 * hf + 2)
                nc.scalar.activation(hh[:, sl], ps_h[:, sl], AF.Tanh)
                nc.vector.tensor_mul(b2[:, sl], zc[:, sl], hh[:, sl])
                nc.vector.tensor_add(h_new[:, sl], c1[:, sl], b2[:, sl])
            h_prev = h_new

        hout = sm.tile([P, KT, BL], FP32, tag="hout", name="hout")
        nc.vector.tensor_copy(hout[:], h_prev[:])
        for kk in range(KT):
            nc.sync.dma_start(hT_out[kk * P:(kk + 1) * P, :], hout[:, kk])

    nc.compile()
    return nc


_NC_CACHE = {}


def _get_nc(masked, use_bf16=True):
    key = (masked, use_bf16)
    if key not in _NC_CACHE:
        _NC_CACHE[key] = build_nc(T, masked=masked, use_bf16=use_bf16)
    return _NC_CACHE[key]


def _w_layout(w):
    # [DIN, D] -> [128, KT*D] with lay[p, kk*D + j] = w[kk*128 + p, j]
    return np.ascontiguousarray(
        w.reshape(KT, P, D).transpose(1, 0, 2).reshape(P, KT * D), dtype=np.float32)


def _b_layout(b):
    return np.ascontiguousarray(b.reshape(MT, P).T, dtype=np.float32)


def make_in_maps(X, W_z, U_z, b_z, W_r, U_r, b_r, W_h, U_h, b_h, mask,
                 masked):
    X = np.asarray(X, dtype=np.float32)
    shared = {"eye": np.eye(P, dtype=np.float32)}
    for g, w, u, b in (("z", W_z, U_z, b_z), ("r", W_r, U_r, b_r),
                       ("h", W_h, U_h, b_h)):
        shared[f"W{g}"] = _w_layout(np.asarray(w, dtype=np.float32))
        shared[f"U{g}"] = _w_layout(np.asarray(u, dtype=np.float32))
        shared[f"b{g}"] = _b_layout(np.asarray(b, dtype=np.float32))

    in_maps = []
    for c in range(NCORES):
        bsl = slice(c * BL, (c + 1) * BL)
        m = dict(shared)
        m["xT"] = np.ascontiguousarray(
            X[bsl].transpose(2, 1, 0).reshape(DIN, T * BL))
        if masked:
            msh = np.zeros((T, BL), dtype=np.float32)
            msh[1:] = np.asarray(mask)[bsl, :T - 1].T.astype(np.float32)
            m["mb"] = np.ascontiguousarray(
                np.tile(msh[:, None, :], (1, P, KT)))
        in_maps.append(m)
    return in_maps


def kernel(X, W_z, U_z, b_z, W_r, U_r, b_r, W_h, U_h, b_h, mask):
    mask = np.asarray(mask)
    masked = not bool(np.all(mask[:, :T - 1] == 1))
    nc = _get_nc(masked)
    in_maps = make_in_maps(X, W_z, U_z, b_z, W_r, U_r, b_r, W_h, U_h, b_h,
                           mask, masked)
    res = run_bass_kernel_spmd(nc, in_maps, core_ids=list(range(NCORES)))
    out = np.empty((B, D), dtype=np.float32)
    for c in range(NCORES):
        out[c * BL:(c + 1) * BL] = res.results[c]["hT_out"].T
    return out
